# revision 4
# baseline (speedup 1.0000x reference)
"""Single-head self-attention (B=4, S=2048, D=1024, fp32) on 8 trn2 NeuronCores.

Sharding: each core owns (batch b = core//2, sequence half h = core%2); the
core computes the partial softmax numerator/denominator over its own 1024
k/v rows for all 2048 queries of its batch, and the host combines halves.

Q and K are never materialized (weight fusion, like BN folding):
  scores[q,j]*1024 = x_q.(G32 @ x_j) + 32*x_j.w + 32*c0,  G32 = 32*Wq Wk^T
(the x_q.(Wq bk) term is constant per query and softmax-invariant -> dropped).

All heavy matmuls run as fp8e4m3 DoubleRow (2 k-tiles per instruction,
0.5 PE cycles/row) in an error-compensated "residual" scheme: every operand
T is split host- or device-side into T_hi = fp8(T), T_lo = fp8(T - T_hi),
and each contraction accumulates three psum passes
  hi*hi + lo*hi + hi*lo        (the lo*lo term is negligible)
which restores ~7-bit mantissa accuracy at 0.75x the fp16 instruction cost.
Stages:
  A: M = G32 @ x^T (own j cols)  -> psum -> (M_hi, M_lo) fp8
     V = x @ (8*Wv)  (own j rows) -> psum -> (V_hi, V_lo) fp8
     bias_j = (x_j.w)/32 + c0/32 - 4   (tiny fp16 matmuls; -4 keeps exp<45)
  B: raw^T[j,q] = 3-pass DR; attn a16 = fp16(exp(raw/1024 + bias)) [ACT],
     a_hi = fp8(a16) [ACT copy], a_lo = a16 - a_hi [DVE]
  C: pre = (a_hi+a_lo)^T V_hi + a_hi^T V_lo   (3-pass DR, psum)
     den = (a_hi+a_lo)^T ones                  (DR, psum)
Outputs pre as fp16 (scaled by 8 via Wv) and den as fp32; the host combines
  out[b] = (pre_h0 + pre_h1) / (8*(den_h0 + den_h1)) + bv
(bv folds out of the device entirely: softmax weights sum to 1).
"""

import numpy as np
import ml_dtypes

import concourse.bass as bass
import concourse.mybir as mybir
import concourse.tile as tile
from concourse.bass_utils import run_bass_kernel_spmd

F8 = mybir.dt.float8e4
F16 = mybir.dt.float16
F32 = mybir.dt.float32
AFT = mybir.ActivationFunctionType
DR = mybir.MatmulPerfMode.DoubleRow

B, S, D = 4, 2048, 1024
NCORES = 8
P = 128
DC = D // P            # 8 contraction chunks of x/G/Wv
CW = DC // 2           # 4 DoubleRow c-windows
JROWS = S // 2         # 1024 own k/v rows per core
JC = JROWS // P        # 8 own j chunks
JW = JC // 2           # 4 DoubleRow j-windows
QB = S // 512          # 4 query col-blocks of 512
SCALE = 1.0 / 32.0     # 1/sqrt(D)
EXPSCALE = 1.0 / 1024.0  # folds G32's *32 and SCALE
SHIFT = 4.0            # exp range shift, cancels in pre/den

NPF8 = ml_dtypes.float8_e4m3

_CACHED = {}


def _split_excess_waits(nc, max_waits=1):
    """walrus in this env rejects >1 sync-wait per instruction (Drain at Tile
    exit carries one per live semaphore); move extras onto same-engine NOPs."""
    for f in nc.m.functions:
        for bb in f.blocks:
            new_list, changed = [], False
            for ins in bb.instructions:
                si = getattr(ins, "sync_info", None)
                ow = list(si.on_wait) if si and si.on_wait else []
                if len(ow) > max_waits:
                    extra, keep = ow[:-max_waits], ow[-max_waits:]
                    for k, w in enumerate(extra):
                        new_list.append(
                            mybir.InstNoOp(
                                name=f"{ins.name}_ws{k}",
                                engine=ins.engine,
                                sync_info=mybir.SyncInfo(on_wait=[w], on_update=[]),
                                bass_nofuse=True,
                            )
                        )
                    si.on_wait = keep
                    changed = True
                new_list.append(ins)
            if changed:
                bb.instructions = new_list


def _build():
    nc = bass.Bass("TRN2", target_bir_lowering=False, debug=False, num_devices=NCORES)

    # xT is the whole batch transposed, own j-half first (host permutes).
    xh_d = nc.dram_tensor("xh", [D, S], F8, kind="ExternalInput").ap()
    xl_d = nc.dram_tensor("xl", [D, S], F8, kind="ExternalInput").ap()
    gh_d = nc.dram_tensor("gh", [D, D], F8, kind="ExternalInput").ap()
    gl_d = nc.dram_tensor("gl", [D, D], F8, kind="ExternalInput").ap()
    wvh_d = nc.dram_tensor("wvh", [D, D], F8, kind="ExternalInput").ap()
    wvl_d = nc.dram_tensor("wvl", [D, D], F8, kind="ExternalInput").ap()
    w_d = nc.dram_tensor("w", [P, DC], F16, kind="ExternalInput").ap()
    c0s_d = nc.dram_tensor("c0s", [P, 1], F32, kind="ExternalInput").ap()
    pre_d = nc.dram_tensor("pre", [S, D], F16, kind="ExternalOutput").ap()
    den_d = nc.dram_tensor("den", [S, 1], F32, kind="ExternalOutput").ap()

    with tile.TileContext(nc) as tc:
        with (
            tc.tile_pool(name="persist", bufs=1) as persist,
            tc.tile_pool(name="attnp", bufs=2) as attnp,
            tc.tile_pool(name="outp", bufs=3) as outp,
            tc.tile_pool(name="small", bufs=4) as small,
        ):
            # ---- persistent SBUF ----
            xh_sb = persist.tile([P, DC, S], F8, tag="xh")
            xl_sb = persist.tile([P, DC, S], F8, tag="xl")
            Mh_sb = persist.tile([P, DC, JROWS], F8, tag="Mh")
            Ml_sb = persist.tile([P, DC, JROWS], F8, tag="Ml")
            Vh_sb = persist.tile([P, JC, D], F8, tag="Vh")
            Vl_sb = persist.tile([P, JC, D], F8, tag="Vl")
            bias_sb = persist.tile([P, JC], F32, tag="bias")
            w_sb = persist.tile([P, DC], F16, tag="w")
            c0s_sb = persist.tile([P, 1], F32, tag="c0s")
            ones_sb = persist.tile([P, 2, 1], F8, tag="ones")

            nc.vector.memset(ones_sb, 1.0)

            # PE warmup: throwaway matmuls while the first DMAs land so the
            # cost model's p-state ramp expires before real work arrives.
            warm_sb = persist.tile([P, 512], F16, tag="warm")
            nc.vector.memset(warm_sb, 0.0)
            with tc.tile_pool(name="psW", bufs=1, space="PSUM") as psW:
                pw = psW.tile([P, 512], F32, tag="psW")
                for _ in range(26):
                    nc.tensor.matmul(
                        pw, warm_sb[:, 0:P], warm_sb, start=True, stop=True
                    )

            # ---- phase A: M, V, bias ----
            with (
                tc.tile_pool(name="pA_in", bufs=1) as pin,
                tc.tile_pool(name="psA", bufs=4, space="PSUM") as psA,
                tc.tile_pool(name="psBias", bufs=2, space="PSUM") as psBias,
            ):
                gh_sb = pin.tile([P, DC, D], F8, tag="gh")
                gl_sb = pin.tile([P, DC, D], F8, tag="gl")
                wvh_sb = pin.tile([P, DC, D], F8, tag="wvh")
                wvl_sb = pin.tile([P, DC, D], F8, tag="wvl")
                # DMA order == consumption order.
                nc.sync.dma_start(out=w_sb, in_=w_d[:, :])
                nc.sync.dma_start(out=c0s_sb, in_=c0s_d[:, :])
                for c in range(DC):
                    cs = slice(c * P, (c + 1) * P)
                    nc.sync.dma_start(out=gh_sb[:, c, :], in_=gh_d[cs, :])
                for c in range(DC):
                    cs = slice(c * P, (c + 1) * P)
                    nc.sync.dma_start(out=gl_sb[:, c, :], in_=gl_d[cs, :])
                for c in range(DC):
                    cs = slice(c * P, (c + 1) * P)
                    nc.sync.dma_start(out=xh_sb[:, c, 0:JROWS], in_=xh_d[cs, 0:JROWS])
                for c in range(DC):
                    cs = slice(c * P, (c + 1) * P)
                    nc.sync.dma_start(out=xl_sb[:, c, 0:JROWS], in_=xl_d[cs, 0:JROWS])
                for c in range(DC):
                    cs = slice(c * P, (c + 1) * P)
                    nc.sync.dma_start(out=wvh_sb[:, c, :], in_=wvh_d[cs, :])
                for c in range(DC):
                    cs = slice(c * P, (c + 1) * P)
                    nc.sync.dma_start(out=wvl_sb[:, c, :], in_=wvl_d[cs, :])
                for c in range(DC):
                    cs = slice(c * P, (c + 1) * P)
                    nc.sync.dma_start(out=xh_sb[:, c, JROWS:S], in_=xh_d[cs, JROWS:S])
                for c in range(DC):
                    cs = slice(c * P, (c + 1) * P)
                    nc.sync.dma_start(out=xl_sb[:, c, JROWS:S], in_=xl_d[cs, JROWS:S])

                # M[d, j] = sum_d' G32[d, d'] x[j, d']  (own j cols).
                # Emit in groups of 4 tiles: hi*hi+lo*hi passes first for all
                # four (these need only gh/gl + xh), then the hi*lo pass, so
                # the PE isn't stalled waiting for the xl DMA.
                def a_m_tile_open(dm, jb):
                    rs = slice(jb * 512, (jb + 1) * 512)
                    ms = slice(dm * P, (dm + 1) * P)
                    ps = psA.tile([P, 512], F32, tag="psA")
                    for cw in range(CW):
                        cs2 = slice(2 * cw, 2 * cw + 2)
                        nc.tensor.matmul(
                            ps, gh_sb[:, cs2, ms], xh_sb[:, cs2, rs],
                            start=(cw == 0), stop=False, perf_mode=DR,
                        )
                    for cw in range(CW):
                        cs2 = slice(2 * cw, 2 * cw + 2)
                        nc.tensor.matmul(
                            ps, gl_sb[:, cs2, ms], xh_sb[:, cs2, rs],
                            start=False, stop=False, perf_mode=DR,
                        )
                    return ps

                def a_m_tile_close(ps, dm, jb):
                    rs = slice(jb * 512, (jb + 1) * 512)
                    ms = slice(dm * P, (dm + 1) * P)
                    for cw in range(CW):
                        cs2 = slice(2 * cw, 2 * cw + 2)
                        nc.tensor.matmul(
                            ps, gh_sb[:, cs2, ms], xl_sb[:, cs2, rs],
                            start=False, stop=(cw == CW - 1), perf_mode=DR,
                        )
                    nc.vector.tensor_copy(Mh_sb[:, dm, rs], ps)
                    nc.vector.tensor_sub(Ml_sb[:, dm, rs], ps, Mh_sb[:, dm, rs])

                tiles = [(dm, jb) for jb in range(JROWS // 512) for dm in range(DC)]
                for g in range(0, len(tiles), 4):
                    grp = tiles[g : g + 4]
                    open_ps = [a_m_tile_open(dm, jb) for dm, jb in grp]
                    for ps, (dm, jb) in zip(open_ps, grp):
                        a_m_tile_close(ps, dm, jb)

                # bias[j] = (x_j.w)*SCALE + c0s  (c0s = c0/32 - SHIFT)
                for j in range(JC):
                    js = slice(j * P, (j + 1) * P)
                    pb = psBias.tile([P, 1], F32, tag="psBias")
                    for c in range(DC):
                        nc.tensor.matmul(
                            pb, xh_sb[:, c, js], w_sb[:, c : c + 1],
                            start=(c == 0), stop=False,
                        )
                    for c in range(DC):
                        nc.tensor.matmul(
                            pb, xl_sb[:, c, js], w_sb[:, c : c + 1],
                            start=False, stop=(c == DC - 1),
                        )
                    nc.vector.tensor_scalar(
                        out=bias_sb[:, j : j + 1], in0=pb,
                        scalar1=float(SCALE), scalar2=c0s_sb[:, 0:1],
                        op0=mybir.AluOpType.mult, op1=mybir.AluOpType.add,
                    )

                # V[j, d] = sum_d' x[j, d'] (8*Wv)[d', d]  (own j rows)
                for j in range(JC):
                    js = slice(j * P, (j + 1) * P)
                    for dh in range(2):
                        os_ = slice(dh * 512, (dh + 1) * 512)
                        ps = psA.tile([P, 512], F32, tag="psA")
                        for cw in range(CW):
                            cs2 = slice(2 * cw, 2 * cw + 2)
                            nc.tensor.matmul(
                                ps, xh_sb[:, cs2, js], wvh_sb[:, cs2, os_],
                                start=(cw == 0), stop=False, perf_mode=DR,
                            )
                        for cw in range(CW):
                            cs2 = slice(2 * cw, 2 * cw + 2)
                            nc.tensor.matmul(
                                ps, xl_sb[:, cs2, js], wvh_sb[:, cs2, os_],
                                start=False, stop=False, perf_mode=DR,
                            )
                        for cw in range(CW):
                            cs2 = slice(2 * cw, 2 * cw + 2)
                            nc.tensor.matmul(
                                ps, xh_sb[:, cs2, js], wvl_sb[:, cs2, os_],
                                start=False, stop=(cw == CW - 1), perf_mode=DR,
                            )
                        nc.vector.tensor_copy(Vh_sb[:, j, os_], ps)
                        nc.vector.tensor_sub(Vl_sb[:, j, os_], ps, Vh_sb[:, j, os_])

            # ---- phases B+C per query block ----
            with (
                tc.tile_pool(name="psB", bufs=2, space="PSUM") as psB,
                tc.tile_pool(name="psC", bufs=4, space="PSUM") as psC,
                tc.tile_pool(name="psD", bufs=2, space="PSUM") as psD,
            ):
                def phase_b(qb):
                    qs = slice(qb * 512, (qb + 1) * 512)
                    a16 = attnp.tile([P, JC, 512], F16, tag="a16")
                    ah = attnp.tile([P, JC, 512], F8, tag="ah")
                    al = attnp.tile([P, JC, 512], F8, tag="al")
                    for j in range(JC):
                        js = slice(j * P, (j + 1) * P)
                        ps = psB.tile([P, 512], F32, tag="psB")
                        for cw in range(CW):
                            cs2 = slice(2 * cw, 2 * cw + 2)
                            nc.tensor.matmul(
                                ps, Mh_sb[:, cs2, js], xh_sb[:, cs2, qs],
                                start=(cw == 0), stop=False, perf_mode=DR,
                            )
                        for cw in range(CW):
                            cs2 = slice(2 * cw, 2 * cw + 2)
                            nc.tensor.matmul(
                                ps, Ml_sb[:, cs2, js], xh_sb[:, cs2, qs],
                                start=False, stop=False, perf_mode=DR,
                            )
                        for cw in range(CW):
                            cs2 = slice(2 * cw, 2 * cw + 2)
                            nc.tensor.matmul(
                                ps, Mh_sb[:, cs2, js], xl_sb[:, cs2, qs],
                                start=False, stop=(cw == CW - 1), perf_mode=DR,
                            )
                        nc.scalar.activation(
                            out=a16[:, j, :], in_=ps, func=AFT.Exp,
                            scale=float(EXPSCALE), bias=bias_sb[:, j : j + 1],
                        )
                        nc.scalar.copy(out=ah[:, j, :], in_=a16[:, j, :])
                        nc.vector.tensor_sub(al[:, j, :], a16[:, j, :], ah[:, j, :])
                    return ah, al

                def phase_c(qb, ah, al):
                    for qc in range(4):
                        qls = slice(qc * P, (qc + 1) * P)
                        qrow = qb * 512 + qc * P
                        pd = psD.tile([P, 1], F32, tag="psD")
                        for jw in range(JW):
                            js2 = slice(2 * jw, 2 * jw + 2)
                            nc.tensor.matmul(
                                pd, ah[:, js2, qls], ones_sb[:, 0:2, 0:1],
                                start=(jw == 0), stop=False, perf_mode=DR,
                            )
                        for jw in range(JW):
                            js2 = slice(2 * jw, 2 * jw + 2)
                            nc.tensor.matmul(
                                pd, al[:, js2, qls], ones_sb[:, 0:2, 0:1],
                                start=False, stop=(jw == JW - 1), perf_mode=DR,
                            )
                        od = small.tile([P, 1], F32, tag="oden")
                        nc.vector.tensor_copy(od, pd)
                        nc.sync.dma_start(out=den_d[qrow : qrow + P, 0:1], in_=od)
                        for dh in range(2):
                            os_ = slice(dh * 512, (dh + 1) * 512)
                            po = psC.tile([P, 512], F32, tag="psC")
                            for jw in range(JW):
                                js2 = slice(2 * jw, 2 * jw + 2)
                                nc.tensor.matmul(
                                    po, ah[:, js2, qls], Vh_sb[:, js2, os_],
                                    start=(jw == 0), stop=False, perf_mode=DR,
                                )
                            for jw in range(JW):
                                js2 = slice(2 * jw, 2 * jw + 2)
                                nc.tensor.matmul(
                                    po, al[:, js2, qls], Vh_sb[:, js2, os_],
                                    start=False, stop=False, perf_mode=DR,
                                )
                            for jw in range(JW):
                                js2 = slice(2 * jw, 2 * jw + 2)
                                nc.tensor.matmul(
                                    po, ah[:, js2, qls], Vl_sb[:, js2, os_],
                                    start=False, stop=(jw == JW - 1), perf_mode=DR,
                                )
                            o = outp.tile([P, 512], F16, tag="o")
                            nc.scalar.copy(out=o, in_=po)
                            nc.sync.dma_start(
                                out=pre_d[qrow : qrow + P, os_], in_=o
                            )

                # B0 B1 C0 B2 C1 B3 C2 C3: keeps the PE a full query-block
                # ahead of the exp/copy chain it depends on.
                pend = []
                for qb in range(QB):
                    pend.append((qb, *phase_b(qb)))
                    if len(pend) == 2:
                        q0, ah0, al0 = pend.pop(0)
                        phase_c(q0, ah0, al0)
                for q0, ah0, al0 in pend:
                    phase_c(q0, ah0, al0)

    _split_excess_waits(nc)
    return nc


def _get_nc():
    if "nc" not in _CACHED:
        _CACHED["nc"] = _build()
    return _CACHED["nc"]


def _hilo(a):
    hi = a.astype(NPF8)
    lo = (a - hi.astype(np.float32)).astype(NPF8)
    return hi, lo


def kernel(x, Wq, bq, Wk, bk, Wv, bv):
    x = np.asarray(x, dtype=np.float32)
    Wq32 = np.asarray(Wq, np.float32)
    Wk32 = np.asarray(Wk, np.float32)
    bq32 = np.asarray(bq, np.float32)
    bk32 = np.asarray(bk, np.float32)
    Wv32 = np.asarray(Wv, np.float32)
    bv32 = np.asarray(bv, np.float32)

    # weight fusion: device needs G32^T = 32*Wk Wq^T (rows = contraction dim)
    gh, gl = _hilo(np.ascontiguousarray(32.0 * (Wk32 @ Wq32.T)))
    wvh, wvl = _hilo(np.ascontiguousarray(8.0 * Wv32))
    w16 = np.ascontiguousarray(
        (Wk32 @ bq32).reshape(DC, P).T
    ).astype(np.float16)
    c0s = np.full((P, 1), float(SCALE) * float(bq32 @ bk32) - SHIFT, np.float32)

    in_maps = []
    for core in range(NCORES):
        b, h = core // 2, core % 2
        # own j rows first (j order is internal; q order is undone on gather)
        xb = np.roll(x[b], -h * JROWS, axis=0) if h else x[b]
        xT = np.ascontiguousarray(xb.T)  # [D, S] fp32
        xh, xl = _hilo(xT)
        in_maps.append(
            {
                "xh": xh, "xl": xl,
                "gh": gh, "gl": gl,
                "wvh": wvh, "wvl": wvl,
                "w": w16, "c0s": c0s,
            }
        )

    res = run_bass_kernel_spmd(_get_nc(), in_maps, list(range(NCORES)))
    out = np.empty((B, S, D), np.float32)
    for b in range(B):
        r0, r1 = res.results[2 * b], res.results[2 * b + 1]
        pre = r0["pre"].astype(np.float32) + np.roll(
            r1["pre"].astype(np.float32), JROWS, axis=0
        )
        den = r0["den"] + np.roll(r1["den"], JROWS, axis=0)
        out[b] = pre / (8.0 * den) + bv32
    return out


# revision 25
# speedup vs baseline: 1.1339x; 1.1339x over previous
"""Single-head self-attention (B=4, S=2048, D=1024, fp32) on 8 trn2 NeuronCores.

Sharding: each core owns (batch b = core//2, sequence half h = core%2); the
core computes the partial softmax numerator/denominator over its own 1024
k/v rows for all 2048 queries of its batch, and the host combines halves.

Q and K are never materialized (weight fusion, like BN folding):
  scores[q,j]*1024 = x_q.(G32 @ x_j) + 32*x_j.w + 32*c0,  G32 = 32*Wq Wk^T
(the x_q.(Wq bk) term is constant per query and softmax-invariant -> dropped).

All heavy matmuls run as fp8e4m3 DoubleRow (2 k-tiles per instruction,
0.5 PE cycles/row) in an error-compensated "residual" scheme: every operand
T is split host- or device-side into T_hi = fp8(T), T_lo = fp8(T - T_hi),
and each contraction accumulates three psum passes
  hi*hi + lo*hi + hi*lo        (the lo*lo term is negligible)
which restores ~7-bit mantissa accuracy at 0.75x the fp16 instruction cost.
Stages:
  A: M = G32 @ x^T (own j cols)  -> psum -> (M_hi, M_lo) fp8
     V = x @ (8*Wv)  (own j rows) -> psum -> (V_hi, V_lo) fp8
     bias_j = (x_j.w)/32 + c0/32 - 4   (tiny fp16 matmuls; -4 keeps exp<45)
  B: raw^T[j,q] = 3-pass DR; attn a16 = fp16(exp(raw/1024 + bias)) [ACT],
     a_hi = fp8(a16) [ACT copy], a_lo = a16 - a_hi [DVE]
  C: pre = (a_hi+a_lo)^T V_hi + a_hi^T V_lo   (3-pass DR, psum)
     den = (a_hi+a_lo)^T ones                  (DR, psum)
Outputs pre as fp16 (scaled by 8 via Wv) and den as fp32; the host combines
  out[b] = (pre_h0 + pre_h1) / (8*(den_h0 + den_h1)) + bv
(bv folds out of the device entirely: softmax weights sum to 1).
"""

import numpy as np
import ml_dtypes

import concourse.bass as bass
import concourse.mybir as mybir
import concourse.tile as tile
from concourse.bass_utils import run_bass_kernel_spmd

F8 = mybir.dt.float8e4
F16 = mybir.dt.float16
F32 = mybir.dt.float32
AFT = mybir.ActivationFunctionType
DR = mybir.MatmulPerfMode.DoubleRow

B, S, D = 4, 2048, 1024
NCORES = 8
P = 128
DC = D // P            # 8 contraction chunks of x/G/Wv
CW = DC // 2           # 4 DoubleRow c-windows
JROWS = S // 2         # 1024 own k/v rows per core
JC = JROWS // P        # 8 own j chunks
JW = JC // 2           # 4 DoubleRow j-windows
QB = S // 512          # 4 query col-blocks of 512
SCALE = 1.0 / 32.0     # 1/sqrt(D)
EXPSCALE = 1.0 / 1024.0  # folds G32's *32 and SCALE
SHIFT = 4.0            # exp range shift, cancels in pre/den

NPF8 = ml_dtypes.float8_e4m3

_CACHED = {}


def _split_excess_waits(nc, max_waits=1):
    """walrus in this env rejects >1 sync-wait per instruction (Drain at Tile
    exit carries one per live semaphore); move extras onto same-engine NOPs."""
    for f in nc.m.functions:
        for bb in f.blocks:
            new_list, changed = [], False
            for ins in bb.instructions:
                si = getattr(ins, "sync_info", None)
                ow = list(si.on_wait) if si and si.on_wait else []
                if len(ow) > max_waits:
                    extra, keep = ow[:-max_waits], ow[-max_waits:]
                    for k, w in enumerate(extra):
                        new_list.append(
                            mybir.InstNoOp(
                                name=f"{ins.name}_ws{k}",
                                engine=ins.engine,
                                sync_info=mybir.SyncInfo(on_wait=[w], on_update=[]),
                                bass_nofuse=True,
                            )
                        )
                    si.on_wait = keep
                    changed = True
                new_list.append(ins)
            if changed:
                bb.instructions = new_list


def _build():
    nc = bass.Bass("TRN2", target_bir_lowering=False, debug=False, num_devices=NCORES)

    # xT is the whole batch transposed, own j-half first (host permutes).
    xh_d = nc.dram_tensor("xh", [D, S], F8, kind="ExternalInput").ap()
    xl_d = nc.dram_tensor("xl", [D, S], F8, kind="ExternalInput").ap()
    gh_d = nc.dram_tensor("gh", [D, D], F8, kind="ExternalInput").ap()
    gl_d = nc.dram_tensor("gl", [D, D], F8, kind="ExternalInput").ap()
    wvh_d = nc.dram_tensor("wvh", [D, D], F8, kind="ExternalInput").ap()
    wvl_d = nc.dram_tensor("wvl", [D, D], F8, kind="ExternalInput").ap()
    w_d = nc.dram_tensor("w", [P, DC], F16, kind="ExternalInput").ap()
    c0s_d = nc.dram_tensor("c0s", [P, 1], F32, kind="ExternalInput").ap()
    pre_d = nc.dram_tensor("pre", [S, D], F16, kind="ExternalOutput").ap()
    den_d = nc.dram_tensor("den", [S, 1], F32, kind="ExternalOutput").ap()

    with tile.TileContext(nc) as tc:
        with (
            tc.tile_pool(name="persist", bufs=1) as persist,
            tc.tile_pool(name="attnp", bufs=2) as attnp,
            tc.tile_pool(name="outp", bufs=3) as outp,
            tc.tile_pool(name="small", bufs=4) as small,
        ):
            # ---- persistent SBUF ----
            xh_sb = persist.tile([P, DC, S], F8, tag="xh")
            xl_sb = persist.tile([P, DC, S], F8, tag="xl")
            Mh_sb = persist.tile([P, DC, JROWS], F8, tag="Mh")
            Ml_sb = persist.tile([P, DC, JROWS], F8, tag="Ml")
            Vh_sb = persist.tile([P, JC, D], F8, tag="Vh")
            Vl_sb = persist.tile([P, JC, D], F8, tag="Vl")
            bias_sb = persist.tile([P, JC], F32, tag="bias")
            den_sb = persist.tile([P, QB * 4], F32, tag="den")
            w_sb = persist.tile([P, DC], F16, tag="w")
            c0s_sb = persist.tile([P, 1], F32, tag="c0s")
            ones_sb = persist.tile([P, 2, 1], F8, tag="ones")

            nc.vector.memset(ones_sb, 1.0)

            # PE warmup: throwaway matmuls while the first DMAs land so the
            # cost model's p-state ramp expires before real work arrives.
            warm_sb = persist.tile([P, 512], F16, tag="warm")
            nc.vector.memset(warm_sb, 0.0)

            # ---- phase A: M, V, bias ----
            # psW (right side, 1 bank) stays open through phase A; filler
            # matmuls on it are always-ready so the scheduler slots them
            # into the DMA-wait gaps, keeping the p-state ramp hot.
            with (
                tc.tile_pool(name="psW", bufs=1, space="PSUM", side="right") as psW,
                tc.tile_pool(name="pA_in", bufs=1) as pin,
                tc.tile_pool(name="psA", bufs=6, space="PSUM") as psA,
            ):
                pw = psW.tile([P, 512], F32, tag="psW")
                for _ in range(9):
                    nc.tensor.matmul(
                        pw, warm_sb[:, 0:P], warm_sb, start=True, stop=True
                    )
                gh_sb = pin.tile([P, DC, D], F8, tag="gh")
                gl_sb = pin.tile([P, DC, D], F8, tag="gl")
                wvh_sb = pin.tile([P, DC, D], F8, tag="wvh")
                wvl_sb = pin.tile([P, DC, D], F8, tag="wvl")
                # DMA order == consumption order. Each tensor goes in ONE
                # 3D-AP transfer (HWDGE fixed cost is per instruction).
                def chunked(dram_ap, ncols, col0=0):
                    # [p, c, s] <- dram[c*128 + p, col0 + s]
                    return bass.AP(
                        tensor=dram_ap.tensor,
                        offset=dram_ap.offset + col0,
                        ap=[
                            [dram_ap.ap[0][0], P],
                            [dram_ap.ap[0][0] * P, DC],
                            [1, ncols],
                        ],
                    )

                def chunked_h(dram_ap, ncols, chalf, col0=0):
                    # [p, c in half, s] <- dram[(chalf*4 + c)*128 + p, col0+s]
                    return bass.AP(
                        tensor=dram_ap.tensor,
                        offset=dram_ap.offset + col0
                        + chalf * 4 * P * dram_ap.ap[0][0],
                        ap=[
                            [dram_ap.ap[0][0], P],
                            [dram_ap.ap[0][0] * P, DC // 2],
                            [1, ncols],
                        ],
                    )

                # first A_M pass needs gh+xh: stream them in c-halves so the
                # PE can start before the full tensors land
                nc.sync.dma_start(out=gh_sb[:, 0:4, :], in_=chunked_h(gh_d, D, 0))
                nc.sync.dma_start(
                    out=xh_sb[:, 0:4, 0:JROWS], in_=chunked_h(xh_d, JROWS, 0)
                )
                nc.sync.dma_start(out=gh_sb[:, 4:8, :], in_=chunked_h(gh_d, D, 1))
                nc.sync.dma_start(
                    out=xh_sb[:, 4:8, 0:JROWS], in_=chunked_h(xh_d, JROWS, 1)
                )
                nc.sync.dma_start(out=gl_sb, in_=chunked(gl_d, D))
                nc.sync.dma_start(out=xl_sb[:, :, 0:JROWS], in_=chunked(xl_d, JROWS))
                nc.sync.dma_start(out=w_sb, in_=w_d[:, :])
                nc.sync.dma_start(out=c0s_sb, in_=c0s_d[:, :])
                nc.sync.dma_start(out=wvh_sb, in_=chunked(wvh_d, D))
                nc.sync.dma_start(out=wvl_sb, in_=chunked(wvl_d, D))
                nc.sync.dma_start(
                    out=xh_sb[:, :, JROWS:S], in_=chunked(xh_d, JROWS, JROWS)
                )
                nc.sync.dma_start(
                    out=xl_sb[:, :, JROWS:S], in_=chunked(xl_d, JROWS, JROWS)
                )

                # M[d, j] = sum_d' G32[d, d'] x[j, d']  (own j cols).
                # Emit in groups of 4 tiles: hi*hi+lo*hi passes first for all
                # four (these need only gh/gl + xh), then the hi*lo pass, so
                # the PE isn't stalled waiting for the xl DMA.
                def a_m_tile_open(dm, jb):
                    rs = slice(jb * 512, (jb + 1) * 512)
                    ms = slice(dm * P, (dm + 1) * P)
                    ps = psA.tile([P, 512], F32, tag="psA")
                    for cw in range(CW):
                        cs2 = slice(2 * cw, 2 * cw + 2)
                        nc.tensor.matmul(
                            ps, gh_sb[:, cs2, ms], xh_sb[:, cs2, rs],
                            start=(cw == 0), stop=False, perf_mode=DR,
                        )
                    for cw in range(CW):
                        cs2 = slice(2 * cw, 2 * cw + 2)
                        nc.tensor.matmul(
                            ps, gl_sb[:, cs2, ms], xh_sb[:, cs2, rs],
                            start=False, stop=False, perf_mode=DR,
                        )
                    return ps

                def a_m_tile_close(ps, dm, jb):
                    rs = slice(jb * 512, (jb + 1) * 512)
                    ms = slice(dm * P, (dm + 1) * P)
                    for cw in range(CW):
                        cs2 = slice(2 * cw, 2 * cw + 2)
                        nc.tensor.matmul(
                            ps, gh_sb[:, cs2, ms], xl_sb[:, cs2, rs],
                            start=False, stop=(cw == CW - 1), perf_mode=DR,
                        )
                    nc.scalar.copy(out=Mh_sb[:, dm, rs], in_=ps)
                    nc.vector.tensor_sub(Ml_sb[:, dm, rs], ps, Mh_sb[:, dm, rs])

                # Sliding window of 4 open psum groups: close the oldest as
                # each new tile opens, so psum-release latency hides behind
                # three other tiles' matmuls.
                tiles = [(dm, jb) for jb in range(JROWS // 512) for dm in range(DC)]
                window = []
                for dm, jb in tiles:
                    if len(window) == 4:
                        ps0, dm0, jb0 = window.pop(0)
                        a_m_tile_close(ps0, dm0, jb0)
                    window.append((a_m_tile_open(dm, jb), dm, jb))
                for ps0, dm0, jb0 in window:
                    a_m_tile_close(ps0, dm0, jb0)

                # V[j, d] = sum_d' x[j, d'] (8*Wv)[d', d]  (own j rows)
                for j in range(JC):
                    js = slice(j * P, (j + 1) * P)
                    for dh in range(2):
                        os_ = slice(dh * 512, (dh + 1) * 512)
                        ps = psA.tile([P, 512], F32, tag="psA")
                        for cw in range(CW):
                            cs2 = slice(2 * cw, 2 * cw + 2)
                            nc.tensor.matmul(
                                ps, xh_sb[:, cs2, js], wvh_sb[:, cs2, os_],
                                start=(cw == 0), stop=False, perf_mode=DR,
                            )
                        for cw in range(CW):
                            cs2 = slice(2 * cw, 2 * cw + 2)
                            nc.tensor.matmul(
                                ps, xl_sb[:, cs2, js], wvh_sb[:, cs2, os_],
                                start=False, stop=False, perf_mode=DR,
                            )
                        for cw in range(CW):
                            cs2 = slice(2 * cw, 2 * cw + 2)
                            nc.tensor.matmul(
                                ps, xh_sb[:, cs2, js], wvl_sb[:, cs2, os_],
                                start=False, stop=(cw == CW - 1), perf_mode=DR,
                            )
                        nc.scalar.copy(out=Vh_sb[:, j, os_], in_=ps)
                        nc.vector.tensor_sub(Vl_sb[:, j, os_], ps, Vh_sb[:, j, os_])

                # bias[j] = (x_j.w)*SCALE + c0s  (c0s = c0/32 - SHIFT).
                # Emitted last: it borrows psA's [P,512] tiles (using col 0)
                # and its short matmuls pad the PE while A_V's final psum
                # consumers drain, just before phase B begins.
                for j in range(JC):
                    js = slice(j * P, (j + 1) * P)
                    pbt = psA.tile([P, 512], F32, tag="psA")
                    pb = pbt[:, 0:1]
                    for c in range(DC):
                        nc.tensor.matmul(
                            pb, xh_sb[:, c, js], w_sb[:, c : c + 1],
                            start=(c == 0), stop=False,
                        )
                    for c in range(DC):
                        nc.tensor.matmul(
                            pb, xl_sb[:, c, js], w_sb[:, c : c + 1],
                            start=False, stop=(c == DC - 1),
                        )
                    nc.vector.tensor_scalar(
                        out=bias_sb[:, j : j + 1], in0=pb,
                        scalar1=float(SCALE), scalar2=c0s_sb[:, 0:1],
                        op0=mybir.AluOpType.mult, op1=mybir.AluOpType.add,
                    )

            # ---- phases B+C per query block ----
            with (
                tc.tile_pool(name="psB", bufs=2, space="PSUM", side="right") as psB,
                tc.tile_pool(name="psC", bufs=4, space="PSUM") as psC,
                tc.tile_pool(name="psD", bufs=2, space="PSUM") as psD,
            ):
                def phase_b(qb):
                    qs = slice(qb * 512, (qb + 1) * 512)
                    a16 = attnp.tile([P, JC, 512], F16, tag="a16")
                    ah = attnp.tile([P, JC, 512], F8, tag="ah")
                    al = attnp.tile([P, JC, 512], F8, tag="al")
                    for j in range(JC):
                        js = slice(j * P, (j + 1) * P)
                        ps = psB.tile([P, 512], F32, tag="psB")
                        for cw in range(CW):
                            cs2 = slice(2 * cw, 2 * cw + 2)
                            nc.tensor.matmul(
                                ps, Mh_sb[:, cs2, js], xh_sb[:, cs2, qs],
                                start=(cw == 0), stop=False, perf_mode=DR,
                            )
                        for cw in range(CW):
                            cs2 = slice(2 * cw, 2 * cw + 2)
                            nc.tensor.matmul(
                                ps, Ml_sb[:, cs2, js], xh_sb[:, cs2, qs],
                                start=False, stop=False, perf_mode=DR,
                            )
                        for cw in range(CW):
                            cs2 = slice(2 * cw, 2 * cw + 2)
                            nc.tensor.matmul(
                                ps, Mh_sb[:, cs2, js], xl_sb[:, cs2, qs],
                                start=False, stop=(cw == CW - 1), perf_mode=DR,
                            )
                        nc.scalar.activation(
                            out=a16[:, j, :], in_=ps, func=AFT.Exp,
                            scale=float(EXPSCALE), bias=bias_sb[:, j : j + 1],
                        )
                        nc.scalar.copy(out=ah[:, j, :], in_=a16[:, j, :])
                        nc.vector.tensor_sub(al[:, j, :], a16[:, j, :], ah[:, j, :])
                    return ah, al

                def phase_c(qb, ah, al):
                    for qc in range(4):
                        qls = slice(qc * P, (qc + 1) * P)
                        qrow = qb * 512 + qc * P
                        pd = psD.tile([P, 1], F32, tag="psD")
                        for jw in range(JW):
                            js2 = slice(2 * jw, 2 * jw + 2)
                            nc.tensor.matmul(
                                pd, ah[:, js2, qls], ones_sb[:, 0:2, 0:1],
                                start=(jw == 0), stop=False, perf_mode=DR,
                            )
                        for jw in range(JW):
                            js2 = slice(2 * jw, 2 * jw + 2)
                            nc.tensor.matmul(
                                pd, al[:, js2, qls], ones_sb[:, 0:2, 0:1],
                                start=False, stop=(jw == JW - 1), perf_mode=DR,
                            )
                        t = qb * 4 + qc
                        nc.vector.tensor_copy(den_sb[:, t : t + 1], pd)
                        for dh in range(2):
                            os_ = slice(dh * 512, (dh + 1) * 512)
                            po = psC.tile([P, 512], F32, tag="psC")
                            for jw in range(JW):
                                js2 = slice(2 * jw, 2 * jw + 2)
                                nc.tensor.matmul(
                                    po, ah[:, js2, qls], Vh_sb[:, js2, os_],
                                    start=(jw == 0), stop=False, perf_mode=DR,
                                )
                            for jw in range(JW):
                                js2 = slice(2 * jw, 2 * jw + 2)
                                nc.tensor.matmul(
                                    po, al[:, js2, qls], Vh_sb[:, js2, os_],
                                    start=False, stop=False, perf_mode=DR,
                                )
                            for jw in range(JW):
                                js2 = slice(2 * jw, 2 * jw + 2)
                                nc.tensor.matmul(
                                    po, ah[:, js2, qls], Vl_sb[:, js2, os_],
                                    start=False, stop=(jw == JW - 1), perf_mode=DR,
                                )
                            o = outp.tile([P, 512], F16, tag="o")
                            nc.vector.tensor_copy(o, po)
                            nc.sync.dma_start(
                                out=pre_d[qrow : qrow + P, os_], in_=o
                            )

                # B0 B1 C0 B2 C1 B3 C2 C3: keeps the PE a full query-block
                # ahead of the exp/copy chain it depends on.
                pend = []
                for qb in range(QB):
                    pend.append((qb, *phase_b(qb)))
                    if len(pend) == 2:
                        q0, ah0, al0 = pend.pop(0)
                        phase_c(q0, ah0, al0)
                for q0, ah0, al0 in pend:
                    phase_c(q0, ah0, al0)

                # den[t*128 + p] <- den_sb[p, t], one transfer
                den_out = bass.AP(
                    tensor=den_d.tensor,
                    offset=den_d.offset,
                    ap=[[1, P], [P, QB * 4]],
                )
                nc.sync.dma_start(out=den_out, in_=den_sb)

    _split_excess_waits(nc)
    return nc


def _get_nc():
    if "nc" not in _CACHED:
        _CACHED["nc"] = _build()
    return _CACHED["nc"]


def _hilo(a):
    hi = a.astype(NPF8)
    lo = (a - hi.astype(np.float32)).astype(NPF8)
    return hi, lo


def kernel(x, Wq, bq, Wk, bk, Wv, bv):
    x = np.asarray(x, dtype=np.float32)
    Wq32 = np.asarray(Wq, np.float32)
    Wk32 = np.asarray(Wk, np.float32)
    bq32 = np.asarray(bq, np.float32)
    bk32 = np.asarray(bk, np.float32)
    Wv32 = np.asarray(Wv, np.float32)
    bv32 = np.asarray(bv, np.float32)

    # weight fusion: device needs G32^T = 32*Wk Wq^T (rows = contraction dim)
    gh, gl = _hilo(np.ascontiguousarray(32.0 * (Wk32 @ Wq32.T)))
    wvh, wvl = _hilo(np.ascontiguousarray(8.0 * Wv32))
    w16 = np.ascontiguousarray(
        (Wk32 @ bq32).reshape(DC, P).T
    ).astype(np.float16)
    c0s = np.full((P, 1), float(SCALE) * float(bq32 @ bk32) - SHIFT, np.float32)

    in_maps = []
    for core in range(NCORES):
        b, h = core // 2, core % 2
        # own j rows first (j order is internal; q order is undone on gather)
        xb = np.roll(x[b], -h * JROWS, axis=0) if h else x[b]
        xT = np.ascontiguousarray(xb.T)  # [D, S] fp32
        xh, xl = _hilo(xT)
        in_maps.append(
            {
                "xh": xh, "xl": xl,
                "gh": gh, "gl": gl,
                "wvh": wvh, "wvl": wvl,
                "w": w16, "c0s": c0s,
            }
        )

    res = run_bass_kernel_spmd(_get_nc(), in_maps, list(range(NCORES)))
    out = np.empty((B, S, D), np.float32)
    for b in range(B):
        r0, r1 = res.results[2 * b], res.results[2 * b + 1]
        pre = r0["pre"].astype(np.float32) + np.roll(
            r1["pre"].astype(np.float32), JROWS, axis=0
        )
        den = r0["den"] + np.roll(r1["den"], JROWS, axis=0)
        out[b] = pre / (8.0 * den) + bv32
    return out


# revision 30
# speedup vs baseline: 1.1354x; 1.0013x over previous
"""Single-head self-attention (B=4, S=2048, D=1024, fp32) on 8 trn2 NeuronCores.

Sharding: each core owns (batch b = core//2, sequence half h = core%2); the
core computes the partial softmax numerator/denominator over its own 1024
k/v rows for all 2048 queries of its batch, and the host combines halves.

Q and K are never materialized (weight fusion, like BN folding):
  scores[q,j]*1024 = x_q.(G32 @ x_j) + 32*x_j.w + 32*c0,  G32 = 32*Wq Wk^T
(the x_q.(Wq bk) term is constant per query and softmax-invariant -> dropped).

All heavy matmuls run as fp8e4m3 DoubleRow (2 k-tiles per instruction,
0.5 PE cycles/row) in an error-compensated "residual" scheme: every operand
T is split host- or device-side into T_hi = fp8(T), T_lo = fp8(T - T_hi),
and each contraction accumulates three psum passes
  hi*hi + lo*hi + hi*lo        (the lo*lo term is negligible)
which restores ~7-bit mantissa accuracy at 0.75x the fp16 instruction cost.
Stages:
  A: M = G32 @ x^T (own j cols)  -> psum -> (M_hi, M_lo) fp8
     V = x @ (8*Wv)  (own j rows) -> psum -> (V_hi, V_lo) fp8
     bias_j = (x_j.w)/32 + c0/32 - 4   (tiny fp16 matmuls; -4 keeps exp<45)
  B: raw^T[j,q] = 3-pass DR; attn a16 = fp16(exp(raw/1024 + bias)) [ACT],
     a_hi = fp8(a16) [ACT copy], a_lo = a16 - a_hi [DVE]
  C: pre = (a_hi+a_lo)^T V_hi + a_hi^T V_lo   (3-pass DR, psum)
     den = (a_hi+a_lo)^T ones                  (DR, psum)
Outputs pre as fp16 (scaled by 8 via Wv) and den as fp32; the host combines
  out[b] = (pre_h0 + pre_h1) / (8*(den_h0 + den_h1)) + bv
(bv folds out of the device entirely: softmax weights sum to 1).
"""

import numpy as np
import ml_dtypes

import concourse.bass as bass
import concourse.mybir as mybir
import concourse.tile as tile
from concourse.bass_utils import run_bass_kernel_spmd

F8 = mybir.dt.float8e4
F16 = mybir.dt.float16
F32 = mybir.dt.float32
AFT = mybir.ActivationFunctionType
DR = mybir.MatmulPerfMode.DoubleRow

B, S, D = 4, 2048, 1024
NCORES = 8
P = 128
DC = D // P            # 8 contraction chunks of x/G/Wv
CW = DC // 2           # 4 DoubleRow c-windows
JROWS = S // 2         # 1024 own k/v rows per core
JC = JROWS // P        # 8 own j chunks
JW = JC // 2           # 4 DoubleRow j-windows
QB = S // 512          # 4 query col-blocks of 512
SCALE = 1.0 / 32.0     # 1/sqrt(D)
EXPSCALE = 1.0 / 1024.0  # folds G32's *32 and SCALE
SHIFT = 4.0            # exp range shift, cancels in pre/den

NPF8 = ml_dtypes.float8_e4m3

_CACHED = {}


def _split_excess_waits(nc, max_waits=1):
    """walrus in this env rejects >1 sync-wait per instruction (Drain at Tile
    exit carries one per live semaphore); move extras onto same-engine NOPs."""
    for f in nc.m.functions:
        for bb in f.blocks:
            new_list, changed = [], False
            for ins in bb.instructions:
                si = getattr(ins, "sync_info", None)
                ow = list(si.on_wait) if si and si.on_wait else []
                if len(ow) > max_waits:
                    extra, keep = ow[:-max_waits], ow[-max_waits:]
                    for k, w in enumerate(extra):
                        new_list.append(
                            mybir.InstNoOp(
                                name=f"{ins.name}_ws{k}",
                                engine=ins.engine,
                                sync_info=mybir.SyncInfo(on_wait=[w], on_update=[]),
                                bass_nofuse=True,
                            )
                        )
                    si.on_wait = keep
                    changed = True
                new_list.append(ins)
            if changed:
                bb.instructions = new_list


def _build():
    nc = bass.Bass("TRN2", target_bir_lowering=False, debug=False, num_devices=NCORES)

    # xT is the whole batch transposed, own j-half first (host permutes).
    xh_d = nc.dram_tensor("xh", [D, S], F8, kind="ExternalInput").ap()
    xl_d = nc.dram_tensor("xl", [D, S], F8, kind="ExternalInput").ap()
    gh_d = nc.dram_tensor("gh", [D, D], F8, kind="ExternalInput").ap()
    gl_d = nc.dram_tensor("gl", [D, D], F8, kind="ExternalInput").ap()
    wvh_d = nc.dram_tensor("wvh", [D, D], F8, kind="ExternalInput").ap()
    wvl_d = nc.dram_tensor("wvl", [D, D], F8, kind="ExternalInput").ap()
    w_d = nc.dram_tensor("w", [P, DC], F16, kind="ExternalInput").ap()
    c0s_d = nc.dram_tensor("c0s", [P, 1], F32, kind="ExternalInput").ap()
    pre_d = nc.dram_tensor("pre", [S, D], F16, kind="ExternalOutput").ap()
    den_d = nc.dram_tensor("den", [S, 1], F32, kind="ExternalOutput").ap()

    with tile.TileContext(nc) as tc:
        with (
            tc.tile_pool(name="persist", bufs=1) as persist,
            tc.tile_pool(name="attnp", bufs=2) as attnp,
            tc.tile_pool(name="outp", bufs=3) as outp,
            tc.tile_pool(name="small", bufs=4) as small,
        ):
            # ---- persistent SBUF ----
            xh_sb = persist.tile([P, DC, S], F8, tag="xh")
            xl_sb = persist.tile([P, DC, S], F8, tag="xl")
            Mh_sb = persist.tile([P, DC, JROWS], F8, tag="Mh")
            Ml_sb = persist.tile([P, DC, JROWS], F8, tag="Ml")
            Vh_sb = persist.tile([P, JC, D], F8, tag="Vh")
            Vl_sb = persist.tile([P, JC, D], F8, tag="Vl")
            bias_sb = persist.tile([P, JC], F32, tag="bias")
            den_sb = persist.tile([P, QB * 4], F32, tag="den")
            w_sb = persist.tile([P, DC], F16, tag="w")
            c0s_sb = persist.tile([P, 1], F32, tag="c0s")
            ones_sb = persist.tile([P, 2, 1], F8, tag="ones")

            nc.vector.memset(ones_sb, 1.0)

            # PE warmup: throwaway matmuls while the first DMAs land so the
            # cost model's p-state ramp expires before real work arrives.
            warm_sb = persist.tile([P, 512], F16, tag="warm")
            nc.vector.memset(warm_sb, 0.0)

            # ---- phase A: M, V, bias ----
            # psW (right side, 1 bank) stays open through phase A; filler
            # matmuls on it are always-ready so the scheduler slots them
            # into the DMA-wait gaps, keeping the p-state ramp hot.
            with (
                tc.tile_pool(name="psW", bufs=1, space="PSUM", side="right") as psW,
                tc.tile_pool(name="pA_in", bufs=1) as pin,
                tc.tile_pool(name="psA", bufs=6, space="PSUM") as psA,
            ):
                pw = psW.tile([P, 512], F32, tag="psW")
                for _ in range(9):
                    nc.tensor.matmul(
                        pw, warm_sb[:, 0:P], warm_sb, start=True, stop=True
                    )
                gh_sb = pin.tile([P, DC, D], F8, tag="gh")
                gl_sb = pin.tile([P, DC, D], F8, tag="gl")
                wvh_sb = pin.tile([P, DC, D], F8, tag="wvh")
                wvl_sb = pin.tile([P, DC, D], F8, tag="wvl")
                # DMA order == consumption order. Each tensor goes in ONE
                # 3D-AP transfer (HWDGE fixed cost is per instruction).
                def chunked(dram_ap, ncols, col0=0):
                    # [p, c, s] <- dram[c*128 + p, col0 + s]
                    return bass.AP(
                        tensor=dram_ap.tensor,
                        offset=dram_ap.offset + col0,
                        ap=[
                            [dram_ap.ap[0][0], P],
                            [dram_ap.ap[0][0] * P, DC],
                            [1, ncols],
                        ],
                    )

                def chunked_h(dram_ap, ncols, chalf, col0=0):
                    # [p, c in half, s] <- dram[(chalf*4 + c)*128 + p, col0+s]
                    return bass.AP(
                        tensor=dram_ap.tensor,
                        offset=dram_ap.offset + col0
                        + chalf * 4 * P * dram_ap.ap[0][0],
                        ap=[
                            [dram_ap.ap[0][0], P],
                            [dram_ap.ap[0][0] * P, DC // 2],
                            [1, ncols],
                        ],
                    )

                # first A_M pass needs gh+xh: stream them in c-halves so the
                # PE can start before the full tensors land
                nc.sync.dma_start(out=gh_sb[:, 0:4, :], in_=chunked_h(gh_d, D, 0))
                nc.sync.dma_start(
                    out=xh_sb[:, 0:4, 0:JROWS], in_=chunked_h(xh_d, JROWS, 0)
                )
                nc.sync.dma_start(out=gh_sb[:, 4:8, :], in_=chunked_h(gh_d, D, 1))
                nc.sync.dma_start(
                    out=xh_sb[:, 4:8, 0:JROWS], in_=chunked_h(xh_d, JROWS, 1)
                )
                nc.sync.dma_start(out=gl_sb, in_=chunked(gl_d, D))
                nc.sync.dma_start(out=xl_sb[:, :, 0:JROWS], in_=chunked(xl_d, JROWS))
                nc.sync.dma_start(out=w_sb, in_=w_d[:, :])
                nc.sync.dma_start(out=c0s_sb, in_=c0s_d[:, :])
                nc.sync.dma_start(out=wvh_sb, in_=chunked(wvh_d, D))
                nc.sync.dma_start(out=wvl_sb, in_=chunked(wvl_d, D))
                nc.sync.dma_start(
                    out=xh_sb[:, :, JROWS:S], in_=chunked(xh_d, JROWS, JROWS)
                )
                nc.sync.dma_start(
                    out=xl_sb[:, :, JROWS:S], in_=chunked(xl_d, JROWS, JROWS)
                )

                # M[d, j] = sum_d' G32[d, d'] x[j, d']  (own j cols).
                # Emit in groups of 4 tiles: hi*hi+lo*hi passes first for all
                # four (these need only gh/gl + xh), then the hi*lo pass, so
                # the PE isn't stalled waiting for the xl DMA.
                def a_m_tile_open(dm, jb):
                    rs = slice(jb * 512, (jb + 1) * 512)
                    ms = slice(dm * P, (dm + 1) * P)
                    ps = psA.tile([P, 512], F32, tag="psA")
                    for cw in range(CW):
                        cs2 = slice(2 * cw, 2 * cw + 2)
                        nc.tensor.matmul(
                            ps, gh_sb[:, cs2, ms], xh_sb[:, cs2, rs],
                            start=(cw == 0), stop=False, perf_mode=DR,
                        )
                    for cw in range(CW):
                        cs2 = slice(2 * cw, 2 * cw + 2)
                        nc.tensor.matmul(
                            ps, gl_sb[:, cs2, ms], xh_sb[:, cs2, rs],
                            start=False, stop=False, perf_mode=DR,
                        )
                    return ps

                def a_m_tile_close(ps, dm, jb):
                    rs = slice(jb * 512, (jb + 1) * 512)
                    ms = slice(dm * P, (dm + 1) * P)
                    for cw in range(CW):
                        cs2 = slice(2 * cw, 2 * cw + 2)
                        nc.tensor.matmul(
                            ps, gh_sb[:, cs2, ms], xl_sb[:, cs2, rs],
                            start=False, stop=(cw == CW - 1), perf_mode=DR,
                        )
                    nc.scalar.copy(out=Mh_sb[:, dm, rs], in_=ps)
                    nc.vector.tensor_sub(Ml_sb[:, dm, rs], ps, Mh_sb[:, dm, rs])

                # Sliding window of 4 open psum groups: close the oldest as
                # each new tile opens, so psum-release latency hides behind
                # three other tiles' matmuls.
                tiles = [(dm, jb) for jb in range(JROWS // 512) for dm in range(DC)]
                window = []
                for dm, jb in tiles:
                    if len(window) == 4:
                        ps0, dm0, jb0 = window.pop(0)
                        a_m_tile_close(ps0, dm0, jb0)
                    window.append((a_m_tile_open(dm, jb), dm, jb))
                for ps0, dm0, jb0 in window:
                    a_m_tile_close(ps0, dm0, jb0)

                # V[j, d] = sum_d' x[j, d'] (8*Wv)[d', d]  (own j rows)
                for j in range(JC):
                    js = slice(j * P, (j + 1) * P)
                    for dh in range(2):
                        os_ = slice(dh * 512, (dh + 1) * 512)
                        ps = psA.tile([P, 512], F32, tag="psA")
                        for cw in range(CW):
                            cs2 = slice(2 * cw, 2 * cw + 2)
                            nc.tensor.matmul(
                                ps, xh_sb[:, cs2, js], wvh_sb[:, cs2, os_],
                                start=(cw == 0), stop=False, perf_mode=DR,
                            )
                        for cw in range(CW):
                            cs2 = slice(2 * cw, 2 * cw + 2)
                            nc.tensor.matmul(
                                ps, xl_sb[:, cs2, js], wvh_sb[:, cs2, os_],
                                start=False, stop=False, perf_mode=DR,
                            )
                        for cw in range(CW):
                            cs2 = slice(2 * cw, 2 * cw + 2)
                            nc.tensor.matmul(
                                ps, xh_sb[:, cs2, js], wvl_sb[:, cs2, os_],
                                start=False, stop=(cw == CW - 1), perf_mode=DR,
                            )
                        nc.scalar.copy(out=Vh_sb[:, j, os_], in_=ps)
                        nc.vector.tensor_sub(Vl_sb[:, j, os_], ps, Vh_sb[:, j, os_])

                # bias[j] = (x_j.w)*SCALE + c0s  (c0s = c0/32 - SHIFT).
                # Emitted last: it borrows psA's [P,512] tiles (using col 0)
                # and its short matmuls pad the PE while A_V's final psum
                # consumers drain, just before phase B begins.
                for j in range(JC):
                    js = slice(j * P, (j + 1) * P)
                    pbt = psA.tile([P, 512], F32, tag="psA")
                    pb = pbt[:, 0:1]
                    for c in range(DC):
                        nc.tensor.matmul(
                            pb, xh_sb[:, c, js], w_sb[:, c : c + 1],
                            start=(c == 0), stop=False,
                        )
                    for c in range(DC):
                        nc.tensor.matmul(
                            pb, xl_sb[:, c, js], w_sb[:, c : c + 1],
                            start=False, stop=(c == DC - 1),
                        )
                    nc.vector.tensor_scalar(
                        out=bias_sb[:, j : j + 1], in0=pb,
                        scalar1=float(SCALE), scalar2=c0s_sb[:, 0:1],
                        op0=mybir.AluOpType.mult, op1=mybir.AluOpType.add,
                    )

            # ---- phases B+C per query block ----
            with (
                tc.tile_pool(name="psB", bufs=2, space="PSUM", side="right") as psB,
                tc.tile_pool(name="psC", bufs=5, space="PSUM") as psC,
                tc.tile_pool(name="psD", bufs=1, space="PSUM") as psD,
            ):
                def phase_b(qb):
                    qs = slice(qb * 512, (qb + 1) * 512)
                    a16 = attnp.tile([P, JC, 512], F16, tag="a16")
                    ah = attnp.tile([P, JC, 512], F8, tag="ah")
                    al = attnp.tile([P, JC, 512], F8, tag="al")
                    for j in range(JC):
                        js = slice(j * P, (j + 1) * P)
                        ps = psB.tile([P, 512], F32, tag="psB")
                        for cw in range(CW):
                            cs2 = slice(2 * cw, 2 * cw + 2)
                            nc.tensor.matmul(
                                ps, Mh_sb[:, cs2, js], xh_sb[:, cs2, qs],
                                start=(cw == 0), stop=False, perf_mode=DR,
                            )
                        for cw in range(CW):
                            cs2 = slice(2 * cw, 2 * cw + 2)
                            nc.tensor.matmul(
                                ps, Ml_sb[:, cs2, js], xh_sb[:, cs2, qs],
                                start=False, stop=False, perf_mode=DR,
                            )
                        for cw in range(CW):
                            cs2 = slice(2 * cw, 2 * cw + 2)
                            nc.tensor.matmul(
                                ps, Mh_sb[:, cs2, js], xl_sb[:, cs2, qs],
                                start=False, stop=(cw == CW - 1), perf_mode=DR,
                            )
                        nc.scalar.activation(
                            out=a16[:, j, :], in_=ps, func=AFT.Exp,
                            scale=float(EXPSCALE), bias=bias_sb[:, j : j + 1],
                        )
                        nc.scalar.copy(out=ah[:, j, :], in_=a16[:, j, :])
                        nc.vector.tensor_sub(al[:, j, :], a16[:, j, :], ah[:, j, :])
                    return ah, al

                def phase_c(qb, ah, al):
                    for qc in range(4):
                        qls = slice(qc * P, (qc + 1) * P)
                        qrow = qb * 512 + qc * P
                        pd = psD.tile([P, 1], F32, tag="psD")
                        for jw in range(JW):
                            js2 = slice(2 * jw, 2 * jw + 2)
                            nc.tensor.matmul(
                                pd, ah[:, js2, qls], ones_sb[:, 0:2, 0:1],
                                start=(jw == 0), stop=False, perf_mode=DR,
                            )
                        for jw in range(JW):
                            js2 = slice(2 * jw, 2 * jw + 2)
                            nc.tensor.matmul(
                                pd, al[:, js2, qls], ones_sb[:, 0:2, 0:1],
                                start=False, stop=(jw == JW - 1), perf_mode=DR,
                            )
                        t = qb * 4 + qc
                        nc.scalar.copy(out=den_sb[:, t : t + 1], in_=pd)
                        for dh in range(2):
                            os_ = slice(dh * 512, (dh + 1) * 512)
                            po = psC.tile([P, 512], F32, tag="psC")
                            for jw in range(JW):
                                js2 = slice(2 * jw, 2 * jw + 2)
                                nc.tensor.matmul(
                                    po, ah[:, js2, qls], Vh_sb[:, js2, os_],
                                    start=(jw == 0), stop=False, perf_mode=DR,
                                )
                            for jw in range(JW):
                                js2 = slice(2 * jw, 2 * jw + 2)
                                nc.tensor.matmul(
                                    po, al[:, js2, qls], Vh_sb[:, js2, os_],
                                    start=False, stop=False, perf_mode=DR,
                                )
                            for jw in range(JW):
                                js2 = slice(2 * jw, 2 * jw + 2)
                                nc.tensor.matmul(
                                    po, ah[:, js2, qls], Vl_sb[:, js2, os_],
                                    start=False, stop=(jw == JW - 1), perf_mode=DR,
                                )
                            o = outp.tile([P, 512], F16, tag="o")
                            nc.vector.tensor_copy(o, po)
                            nc.sync.dma_start(
                                out=pre_d[qrow : qrow + P, os_], in_=o
                            )

                # B0 B1 C0 B2 C1 B3 C2 C3: keeps the PE a full query-block
                # ahead of the exp/copy chain it depends on.
                pend = []
                for qb in range(QB):
                    pend.append((qb, *phase_b(qb)))
                    if len(pend) == 2:
                        q0, ah0, al0 = pend.pop(0)
                        phase_c(q0, ah0, al0)
                for q0, ah0, al0 in pend:
                    phase_c(q0, ah0, al0)

                # den[t*128 + p] <- den_sb[p, t], one transfer
                den_out = bass.AP(
                    tensor=den_d.tensor,
                    offset=den_d.offset,
                    ap=[[1, P], [P, QB * 4]],
                )
                nc.sync.dma_start(out=den_out, in_=den_sb)

    _split_excess_waits(nc)
    return nc


def _get_nc():
    if "nc" not in _CACHED:
        _CACHED["nc"] = _build()
    return _CACHED["nc"]


def _hilo(a):
    hi = a.astype(NPF8)
    lo = (a - hi.astype(np.float32)).astype(NPF8)
    return hi, lo


def kernel(x, Wq, bq, Wk, bk, Wv, bv):
    x = np.asarray(x, dtype=np.float32)
    Wq32 = np.asarray(Wq, np.float32)
    Wk32 = np.asarray(Wk, np.float32)
    bq32 = np.asarray(bq, np.float32)
    bk32 = np.asarray(bk, np.float32)
    Wv32 = np.asarray(Wv, np.float32)
    bv32 = np.asarray(bv, np.float32)

    # weight fusion: device needs G32^T = 32*Wk Wq^T (rows = contraction dim)
    gh, gl = _hilo(np.ascontiguousarray(32.0 * (Wk32 @ Wq32.T)))
    wvh, wvl = _hilo(np.ascontiguousarray(8.0 * Wv32))
    w16 = np.ascontiguousarray(
        (Wk32 @ bq32).reshape(DC, P).T
    ).astype(np.float16)
    c0s = np.full((P, 1), float(SCALE) * float(bq32 @ bk32) - SHIFT, np.float32)

    in_maps = []
    for core in range(NCORES):
        b, h = core // 2, core % 2
        # own j rows first (j order is internal; q order is undone on gather)
        xb = np.roll(x[b], -h * JROWS, axis=0) if h else x[b]
        xT = np.ascontiguousarray(xb.T)  # [D, S] fp32
        xh, xl = _hilo(xT)
        in_maps.append(
            {
                "xh": xh, "xl": xl,
                "gh": gh, "gl": gl,
                "wvh": wvh, "wvl": wvl,
                "w": w16, "c0s": c0s,
            }
        )

    res = run_bass_kernel_spmd(_get_nc(), in_maps, list(range(NCORES)))
    out = np.empty((B, S, D), np.float32)
    for b in range(B):
        r0, r1 = res.results[2 * b], res.results[2 * b + 1]
        pre = r0["pre"].astype(np.float32) + np.roll(
            r1["pre"].astype(np.float32), JROWS, axis=0
        )
        den = r0["den"] + np.roll(r1["den"], JROWS, axis=0)
        out[b] = pre / (8.0 * den) + bv32
    return out
